# revision 18
# baseline (speedup 1.0000x reference)
"""LocalizationAttacks kernel for 8 Trainium2 NeuronCores.

Data-parallel over the batch dim: each of the 8 cores processes 4 of the 32
batch items. The per-segment attack decisions (tiny [B, 300] masks) are
precomputed on the host from seg_starts/revert_flags and shipped to the device
as per-partition scalars; the 300 MB of audio streaming (2 input streams,
3 output streams) runs on-device, DMA-bound.

Per core the audio is a flat stream of 1200 segments x 1600 f32, processed
in tiles of [p partitions, K segments per partition row] following PLAN.
Early tiles are small so the store ring starts draining early; later tiles
are wide so writes run at their best rate. Per [p, 1600] slice:
  attacked = wm * (1-am) + og * rm     (tensor_scalar_mul + fused stt)
  update_o = og * (1-zm)               (tensor_scalar_mul)
  ground_t = broadcast(1-am)           (tensor_scalar: wm*0 + mask)
with per-partition [p,1] mask scalars taken from a single mask tile loaded
once up front. All compute runs on DVE so the ACT engine is a pure store
issuer (ring backpressure then never delays compute). Audio loads ride the
SP HWDGE ring, stores the ACT HWDGE ring, except iteration 4's stores which
ride the SP ring after its loads are issued — balancing ring bytes so both
rings drain together (~420 GB/s aggregate, fabric-limited).
"""

import numpy as np

import concourse.bacc as bacc
import concourse.bass as bass
import concourse.mybir as mybir
from concourse.bass_utils import run_bass_kernel_spmd
from concourse.tile import TileContext

# Problem shape (hardcoded per contract)
B, C, T = 32, 1, 480000
SEG = 1600
S = T // SEG              # 300 segments per item
N_CORES = 8
B_LOC = B // N_CORES      # 4 items per core
N_SEGS = B_LOC * S        # 1200 segments per core
P = 128

# (partitions, segments-per-partition-row) per tile; rows sum to N_SEGS
PLAN = [(128, 1), (128, 1), (128, 2), (128, 2), (128, 2), (88, 2)]
assert sum(p * k for p, k in PLAN) == N_SEGS
N_MASK_COLS = 3 * sum(k for _, k in PLAN)

F32 = mybir.dt.float32


def _build_nc() -> bass.Bass:
    nc = bacc.Bacc()
    wm = nc.dram_tensor("wm", [N_SEGS * SEG], F32, kind="ExternalInput")
    og = nc.dram_tensor("og", [N_SEGS * SEG], F32, kind="ExternalInput")
    mk = nc.dram_tensor("mk", [P, N_MASK_COLS], F32, kind="ExternalInput")
    att = nc.dram_tensor("att", [N_SEGS * SEG], F32, kind="ExternalOutput")
    gt = nc.dram_tensor("gt", [N_SEGS * SEG], F32, kind="ExternalOutput")
    uo = nc.dram_tensor("uo", [N_SEGS * SEG], F32, kind="ExternalOutput")

    mult = mybir.AluOpType.mult
    add = mybir.AluOpType.add

    def view(t, e0, p, k):
        return t[e0 : e0 + p * k * SEG].rearrange("(p f) -> p f", p=p)

    with TileContext(nc) as tc:
        with tc.tile_pool(name="io", bufs=2) as pool:
            # all iterations' masks in one tiny tile, loaded once
            m_all = pool.tile([P, N_MASK_COLS], F32, tag="m", bufs=1)
            nc.sync.dma_start(out=m_all[:], in_=mk[:, :])
            ones_t = pool.tile([P, SEG], F32, tag="ones", bufs=1)
            nc.gpsimd.memset(ones_t[:], 1.0)
            pad = [P, 2 * SEG]
            # Pass 1: all loads on the SP HWDGE ring, nothing else in the
            # SP issue stream ahead of the tail stores below.
            in_tiles = []
            e0 = 0
            for p, k in PLAN:
                row = k * SEG
                wm_t = pool.tile([p, row], F32, tag="wm", bufs=3, padded_shape=pad)
                og_t = pool.tile([p, row], F32, tag="og", bufs=3, padded_shape=pad)
                nc.sync.dma_start(out=wm_t[:], in_=view(wm, e0, p, k))
                nc.sync.dma_start(out=og_t[:], in_=view(og, e0, p, k))
                in_tiles.append((wm_t, og_t))
                e0 += p * k * SEG
            # Pass 2a: ground_truth first — it depends only on the 1.9 KB
            # mask tile, so its 7.68 MB of stores saturate the ACT ring from
            # ~9 us while the big loads are still arriving.
            e0 = 0
            off = 0
            for p, k in PLAN:
                row = k * SEG
                gt_t = pool.tile([p, row], F32, tag="gt", bufs=3, padded_shape=pad)
                for j in range(k):
                    sl = slice(j * SEG, (j + 1) * SEG)
                    c = 3 * (off + j)
                    nc.vector.tensor_scalar_mul(
                        gt_t[:, sl], ones_t[:p, :], m_all[:p, c : c + 1]
                    )
                nc.scalar.dma_start(out=view(gt, e0, p, k), in_=gt_t[:])
                e0 += p * k * SEG
                off += k
            # Pass 2b: attacked / update_original (all compute on DVE — ACT
            # stays a pure store issuer so ring backpressure never delays
            # compute). The last two tiles' stores ride the SP ring after
            # its loads, balancing ring bytes ~19.7/18.7 MB.
            e0 = 0
            off = 0
            for it, (p, k) in enumerate(PLAN):
                row = k * SEG
                wm_t, og_t = in_tiles[it]
                at_t = pool.tile([p, row], F32, tag="at", bufs=3, padded_shape=pad)
                uo_t = pool.tile([p, row], F32, tag="uo", bufs=3, padded_shape=pad)
                for j in range(k):
                    sl = slice(j * SEG, (j + 1) * SEG)
                    c = 3 * (off + j)
                    s_am = m_all[:p, c + 0 : c + 1]  # 1 - attack
                    s_rm = m_all[:p, c + 1 : c + 2]  # revert
                    s_zm = m_all[:p, c + 2 : c + 3]  # 1 - zero
                    nc.vector.tensor_scalar_mul(at_t[:, sl], og_t[:, sl], s_rm)
                    nc.vector.scalar_tensor_tensor(
                        at_t[:, sl], wm_t[:, sl], s_am, at_t[:, sl], mult, add
                    )
                    nc.vector.tensor_scalar_mul(uo_t[:, sl], og_t[:, sl], s_zm)
                ring = nc.sync if it >= 4 else nc.scalar
                ring.dma_start(out=view(att, e0, p, k), in_=at_t[:])
                ring.dma_start(out=view(uo, e0, p, k), in_=uo_t[:])
                e0 += p * k * SEG
                off += k
    nc.compile()
    return nc


_NC_CACHE: bass.Bass | None = None


def _pack_masks(oma_rows, rm_rows, omz_rows):
    """Per-core segment masks [N_SEGS] -> one [P, N_MASK_COLS] tile."""
    m_all = np.zeros((P, N_MASK_COLS), np.float32)
    r0 = 0
    off = 0
    for p, k in PLAN:
        for j in range(k):
            c = 3 * (off + j)
            # partition q, slice j holds segment r0 + q*k + j
            m_all[:p, c + 0] = oma_rows[r0 + j : r0 + p * k : k]
            m_all[:p, c + 1] = rm_rows[r0 + j : r0 + p * k : k]
            m_all[:p, c + 2] = omz_rows[r0 + j : r0 + p * k : k]
        r0 += p * k
        off += k
    return m_all


def _prepare_in_maps(original, watermarked, seg_starts, revert_flags):
    original = np.ascontiguousarray(np.asarray(original), dtype=np.float32)
    watermarked = np.ascontiguousarray(np.asarray(watermarked), dtype=np.float32)
    seg_starts = np.asarray(seg_starts)
    revert_flags = np.asarray(revert_flags)

    # Host-side segment masks, [B, 300] each (tiny).
    attack = np.zeros((B, S), np.float32)
    attack[np.arange(B)[:, None], seg_starts] = 1.0
    rf = revert_flags.astype(np.float32)
    one_minus_am = 1.0 - attack
    rm = attack * rf
    one_minus_zm = 1.0 - attack * (1.0 - rf)

    in_maps = []
    for c in range(N_CORES):
        sl = slice(c * B_LOC, (c + 1) * B_LOC)
        in_maps.append(
            {
                "wm": watermarked[sl].reshape(-1),
                "og": original[sl].reshape(-1),
                "mk": _pack_masks(
                    one_minus_am[sl].reshape(-1),
                    rm[sl].reshape(-1),
                    one_minus_zm[sl].reshape(-1),
                ),
            }
        )
    return in_maps


def _gather(results):
    def cat(name):
        return np.concatenate(
            [results[c][name].reshape(B_LOC, C, T) for c in range(N_CORES)], axis=0
        )

    return cat("att"), cat("gt"), cat("uo")


def _run(inputs: dict, **run_kwargs):
    global _NC_CACHE
    if _NC_CACHE is None:
        _NC_CACHE = _build_nc()
    in_maps = _prepare_in_maps(**inputs)
    res = run_bass_kernel_spmd(
        _NC_CACHE, in_maps, core_ids=list(range(N_CORES)), **run_kwargs
    )
    return res, _gather(res.results)


def kernel(original, watermarked, seg_starts, revert_flags):
    _, outs = _run(
        dict(
            original=original,
            watermarked=watermarked,
            seg_starts=seg_starts,
            revert_flags=revert_flags,
        )
    )
    return outs



# revision 22
# speedup vs baseline: 1.1825x; 1.1825x over previous
"""LocalizationAttacks kernel for 8 Trainium2 NeuronCores.

Data-parallel over the batch dim: each of the 8 cores processes 4 of the 32
batch items. The per-segment attack decisions (tiny [B, 300] masks) are
precomputed on the host from seg_starts/revert_flags and shipped to the device
as per-partition scalars; the 300 MB of audio streaming (2 input streams,
3 output streams) runs on-device, DMA-bound.

Per core the audio is a flat stream of 1200 segments x 1600 f32, processed
in tiles of [p partitions, K segments per partition row] following PLAN.
Early tiles are small so the store ring starts draining early; later tiles
are wide so writes run at their best rate. Per [p, 1600] slice:
  attacked = wm * (1-am) + og * rm     (tensor_scalar_mul + fused stt)
  update_o = og * (1-zm)               (tensor_scalar_mul)
  ground_t = broadcast(1-am)           (tensor_scalar: wm*0 + mask)
with per-partition [p,1] mask scalars taken from a single mask tile loaded
once up front. All compute runs on DVE so the ACT engine is a pure store
issuer (ring backpressure then never delays compute). Audio loads ride the
SP HWDGE ring, stores the ACT HWDGE ring, except iteration 4's stores which
ride the SP ring after its loads are issued — balancing ring bytes so both
rings drain together (~420 GB/s aggregate, fabric-limited).
"""

import numpy as np

import concourse.bacc as bacc
import concourse.bass as bass
import concourse.mybir as mybir
from concourse.bass_utils import run_bass_kernel_spmd
from concourse.tile import TileContext

# Problem shape (hardcoded per contract)
B, C, T = 32, 1, 480000
SEG = 1600
S = T // SEG              # 300 segments per item
N_CORES = 8
B_LOC = B // N_CORES      # 4 items per core
N_SEGS = B_LOC * S        # 1200 segments per core
P = 128

# (partitions, segments-per-partition-row) per tile; rows sum to N_SEGS.
# The last three tiles form the tail: t4's stores are split per-slice and
# t5/t6 are small k=1 tiles, so the final stores spread across BOTH HWDGE
# rings (sync 19.5 MB / ACT 18.9 MB) instead of draining on sync alone.
PLAN = [(128, 1), (128, 1), (128, 2), (128, 2), (128, 2), (88, 1), (88, 1)]
assert sum(p * k for p, k in PLAN) == N_SEGS
N_MASK_COLS = 3 * sum(k for _, k in PLAN)

F32 = mybir.dt.float32


def _build_nc() -> bass.Bass:
    nc = bacc.Bacc()
    wm = nc.dram_tensor("wm", [N_SEGS * SEG], F32, kind="ExternalInput")
    og = nc.dram_tensor("og", [N_SEGS * SEG], F32, kind="ExternalInput")
    mk = nc.dram_tensor("mk", [P, N_MASK_COLS], F32, kind="ExternalInput")
    att = nc.dram_tensor("att", [N_SEGS * SEG], F32, kind="ExternalOutput")
    gt = nc.dram_tensor("gt", [N_SEGS * SEG], F32, kind="ExternalOutput")
    uo = nc.dram_tensor("uo", [N_SEGS * SEG], F32, kind="ExternalOutput")

    mult = mybir.AluOpType.mult
    add = mybir.AluOpType.add

    def view(t, e0, p, k):
        return t[e0 : e0 + p * k * SEG].rearrange("(p f) -> p f", p=p)

    with TileContext(nc) as tc:
        with tc.tile_pool(name="io", bufs=2) as pool:
            # all iterations' masks in one tiny tile, loaded once
            m_all = pool.tile([P, N_MASK_COLS], F32, tag="m", bufs=1)
            nc.sync.dma_start(out=m_all[:], in_=mk[:, :])
            ones_t = pool.tile([P, SEG], F32, tag="ones", bufs=1)
            nc.gpsimd.memset(ones_t[:], 1.0)
            pad = [P, 2 * SEG]
            # Pass 1: all loads on the SP HWDGE ring, nothing else in the
            # SP issue stream ahead of the tail stores below.
            in_tiles = []
            e0 = 0
            for p, k in PLAN:
                row = k * SEG
                wm_t = pool.tile([p, row], F32, tag="wm", bufs=3, padded_shape=pad)
                og_t = pool.tile([p, row], F32, tag="og", bufs=3, padded_shape=pad)
                nc.sync.dma_start(out=wm_t[:], in_=view(wm, e0, p, k))
                nc.sync.dma_start(out=og_t[:], in_=view(og, e0, p, k))
                in_tiles.append((wm_t, og_t))
                e0 += p * k * SEG
            # Pass 2a: ground_truth first — it depends only on the 1.9 KB
            # mask tile, so its 7.68 MB of stores saturate the ACT ring from
            # ~9 us while the big loads are still arriving.
            e0 = 0
            off = 0
            for p, k in PLAN:
                row = k * SEG
                gt_t = pool.tile([p, row], F32, tag="gt", bufs=3, padded_shape=pad)
                for j in range(k):
                    sl = slice(j * SEG, (j + 1) * SEG)
                    c = 3 * (off + j)
                    nc.vector.tensor_scalar_mul(
                        gt_t[:, sl], ones_t[:p, :], m_all[:p, c : c + 1]
                    )
                nc.scalar.dma_start(out=view(gt, e0, p, k), in_=gt_t[:])
                e0 += p * k * SEG
                off += k
            # Pass 2b: attacked / update_original (all compute on DVE — ACT
            # stays a pure store issuer so ring backpressure never delays
            # compute). The last two tiles' stores ride the SP ring after
            # its loads, balancing ring bytes ~19.7/18.7 MB.
            e0 = 0
            off = 0
            for it, (p, k) in enumerate(PLAN):
                row = k * SEG
                wm_t, og_t = in_tiles[it]
                at_t = pool.tile([p, row], F32, tag="at", bufs=3, padded_shape=pad)
                uo_t = pool.tile([p, row], F32, tag="uo", bufs=3, padded_shape=pad)
                for j in range(k):
                    sl = slice(j * SEG, (j + 1) * SEG)
                    c = 3 * (off + j)
                    s_am = m_all[:p, c + 0 : c + 1]  # 1 - attack
                    s_rm = m_all[:p, c + 1 : c + 2]  # revert
                    s_zm = m_all[:p, c + 2 : c + 3]  # 1 - zero
                    nc.vector.tensor_scalar_mul(at_t[:, sl], og_t[:, sl], s_rm)
                    nc.vector.scalar_tensor_tensor(
                        at_t[:, sl], wm_t[:, sl], s_am, at_t[:, sl], mult, add
                    )
                    nc.vector.tensor_scalar_mul(uo_t[:, sl], og_t[:, sl], s_zm)
                av = view(att, e0, p, k)
                uv = view(uo, e0, p, k)
                if it < 4:
                    nc.scalar.dma_start(out=av[:], in_=at_t[:])
                    nc.scalar.dma_start(out=uv[:], in_=uo_t[:])
                elif it == 4:
                    # tail starts: split t4's stores into per-slice pieces
                    nc.sync.dma_start(out=av[:, :SEG], in_=at_t[:, :SEG])
                    nc.sync.dma_start(out=av[:, SEG : 2 * SEG],
                                      in_=at_t[:, SEG : 2 * SEG])
                    nc.sync.dma_start(out=uv[:, :SEG], in_=uo_t[:, :SEG])
                    nc.scalar.dma_start(out=uv[:, SEG : 2 * SEG],
                                        in_=uo_t[:, SEG : 2 * SEG])
                elif it == 5:
                    nc.sync.dma_start(out=av[:], in_=at_t[:])
                    nc.sync.dma_start(out=uv[:], in_=uo_t[:])
                else:
                    nc.sync.dma_start(out=av[:], in_=at_t[:])
                    nc.scalar.dma_start(out=uv[:], in_=uo_t[:])
                e0 += p * k * SEG
                off += k
    nc.compile()
    return nc


_NC_CACHE: bass.Bass | None = None


def _pack_masks(oma_rows, rm_rows, omz_rows):
    """Per-core segment masks [N_SEGS] -> one [P, N_MASK_COLS] tile."""
    m_all = np.zeros((P, N_MASK_COLS), np.float32)
    r0 = 0
    off = 0
    for p, k in PLAN:
        for j in range(k):
            c = 3 * (off + j)
            # partition q, slice j holds segment r0 + q*k + j
            m_all[:p, c + 0] = oma_rows[r0 + j : r0 + p * k : k]
            m_all[:p, c + 1] = rm_rows[r0 + j : r0 + p * k : k]
            m_all[:p, c + 2] = omz_rows[r0 + j : r0 + p * k : k]
        r0 += p * k
        off += k
    return m_all


def _prepare_in_maps(original, watermarked, seg_starts, revert_flags):
    original = np.ascontiguousarray(np.asarray(original), dtype=np.float32)
    watermarked = np.ascontiguousarray(np.asarray(watermarked), dtype=np.float32)
    seg_starts = np.asarray(seg_starts)
    revert_flags = np.asarray(revert_flags)

    # Host-side segment masks, [B, 300] each (tiny).
    attack = np.zeros((B, S), np.float32)
    attack[np.arange(B)[:, None], seg_starts] = 1.0
    rf = revert_flags.astype(np.float32)
    one_minus_am = 1.0 - attack
    rm = attack * rf
    one_minus_zm = 1.0 - attack * (1.0 - rf)

    in_maps = []
    for c in range(N_CORES):
        sl = slice(c * B_LOC, (c + 1) * B_LOC)
        in_maps.append(
            {
                "wm": watermarked[sl].reshape(-1),
                "og": original[sl].reshape(-1),
                "mk": _pack_masks(
                    one_minus_am[sl].reshape(-1),
                    rm[sl].reshape(-1),
                    one_minus_zm[sl].reshape(-1),
                ),
            }
        )
    return in_maps


def _gather(results):
    def cat(name):
        return np.concatenate(
            [results[c][name].reshape(B_LOC, C, T) for c in range(N_CORES)], axis=0
        )

    return cat("att"), cat("gt"), cat("uo")


def _run(inputs: dict, **run_kwargs):
    global _NC_CACHE
    if _NC_CACHE is None:
        _NC_CACHE = _build_nc()
    in_maps = _prepare_in_maps(**inputs)
    res = run_bass_kernel_spmd(
        _NC_CACHE, in_maps, core_ids=list(range(N_CORES)), **run_kwargs
    )
    return res, _gather(res.results)


def kernel(original, watermarked, seg_starts, revert_flags):
    _, outs = _run(
        dict(
            original=original,
            watermarked=watermarked,
            seg_starts=seg_starts,
            revert_flags=revert_flags,
        )
    )
    return outs

